# revision 1
# baseline (speedup 1.0000x reference)
"""Trainium2 Bass kernel for out = x @ W.T + b  (x:[8192,1024] f32, W:[1024,1024] f32, b:[1024] f32).

Data-parallel over batch across 8 NeuronCores: each core computes a
[1024,1024] @ [1024,1024]^T matmul + bias for its 1024-row batch shard.

Host-side prep (inside kernel(), not on device): shard x along batch,
pre-transpose x and W so the contraction dim (in_f) lands on SBUF
partitions with fully-contiguous per-partition DMA reads, and cast to
the compute dtype. The PE contracts over the partition dim and fp32 has
no DMA-transpose path, so the host-side layout removes all on-chip
transposes.

Compute modes (MODE):
  f16   : single-pass fp16 matmuls. rel err ~3e-4, fastest DMA (2B in).
  f16x3 : fp16 hi/lo split, 3 accumulated matmuls -> ~fp32 accuracy (~3e-7).
  f32r  : float32r (relaxed fp32) matmuls, 1 cyc/row.
  f32   : exact fp32 matmuls (4 cyc/row), reference-grade.
"""

import os

import numpy as np

import concourse.bass as bass
import concourse.mybir as mybir
import concourse.tile as tile
from concourse import bacc
from concourse.bass_utils import run_bass_kernel_spmd

N_CORES = 8
B, IN_F, OUT_F = 8192, 1024, 1024
B_SHARD = B // N_CORES          # 1024 batch rows per core
P = 128                         # SBUF partitions
KO = IN_F // P                  # 8 contraction subtiles
NT = B_SHARD // P               # 8 batch tiles per core
NO = 2                          # 2 output column tiles of 512
OW = OUT_F // NO                # 512 (one PSUM bank of fp32)

MODE = os.environ.get("BASS_KERNEL_MODE", "f16")

_nc_cache = {}


def _build(mode, skip_out=False, skip_dve=False):
    f32 = mybir.dt.float32
    dt_in = {
        "f16": mybir.dt.float16,
        "f16x3": mybir.dt.float16,
        "f32r": mybir.dt.float32r,
        "f32": f32,
    }[mode]
    split = mode == "f16x3"

    nc = bacc.Bacc("TRN2", target_bir_lowering=False)

    # DRAM layouts are host-packed so every DMA is contiguous per partition:
    #   xt[ki, t, ko, bi]  = x_shard[t*128+bi, ko*128+ki]
    #   wt[ki, ot, ko, oi] = W[ot*512+oi, ko*128+ki]
    xt_d = nc.dram_tensor("xt", [P, NT, KO, P], dt_in, kind="ExternalInput")
    wt_d = nc.dram_tensor("wt", [P, NO, KO, OW], dt_in, kind="ExternalInput")
    if split:
        xl_d = nc.dram_tensor("xl", [P, NT, KO, P], dt_in, kind="ExternalInput")
        wl_d = nc.dram_tensor("wl", [P, NO, KO, OW], dt_in, kind="ExternalInput")
    bias_d = nc.dram_tensor("bias", [1, OUT_F], f32, kind="ExternalInput")
    out_d = nc.dram_tensor("out", [B_SHARD, OUT_F], f32, kind="ExternalOutput")

    # Loop structure: o-half OUTER, batch-tile inner. The first half only
    # needs w0 (1 MB) so the PE starts ~1.6us in; x tiles load once during
    # the first half and are reused from SBUF in the second. Input DMAs ride
    # the SP HWDGE ring (nc.sync); bias + output DMAs ride the ACT ring
    # (nc.scalar) so outputs never FIFO-block input prefetch.
    WCH = 2   # ko per W DMA chunk (256 KB) for w1
    WCH0 = 1  # finer 128 KB chunks for w0 (feeds the cold-start MM stream)
    with tile.TileContext(nc) as tc:
        with (
            tc.tile_pool(name="singles", bufs=1) as singles,
            tc.tile_pool(name="wpool", bufs=NO * (2 if split else 1)) as wpool,
            tc.tile_pool(name="xpool", bufs=NT) as xpool,
            tc.tile_pool(name="xlpool", bufs=NT) as xlpool,
            tc.tile_pool(name="opool", bufs=NT) as opool,
            tc.tile_pool(name="psums", bufs=6, space="PSUM") as psums,
        ):
            bias_sb = singles.tile([P, OUT_F], f32)

            w_tiles = [
                wpool.tile([P, KO, OW], dt_in, name=f"w_{ot}", tag="w_sb")
                for ot in range(NO)
            ]
            wl_tiles = (
                [wpool.tile([P, KO, OW], dt_in, name=f"wl_{ot}", tag="w_sb")
                 for ot in range(NO)] if split else []
            )

            def load_w(ot, kc, lo=False, wch=WCH):
                dst = (wl_tiles if lo else w_tiles)[ot]
                src = (wl_d if lo else wt_d)
                nc.sync.dma_start(
                    out=dst[:, kc:kc + wch], in_=src[:, ot, kc:kc + wch]
                )

            x_tiles, xl_tiles, o_tiles = [], [], []
            for t in range(NT):
                x_sb = xpool.tile([P, KO, P], dt_in, name=f"x_{t}", tag="x_sb")
                x_tiles.append(x_sb)
                xl_tiles.append(
                    xlpool.tile([P, KO, P], dt_in, name=f"xl_{t}", tag="xl_sb")
                    if split else None
                )
                o_tiles.append([
                    opool.tile([P, OW], f32, name=f"o_{t}_{ot}", tag="o_sb")
                    for ot in range(NO)
                ])

            def load_x(t):
                nc.sync.dma_start(out=x_tiles[t][:], in_=xt_d[:, t])
                if split:
                    nc.sync.dma_start(out=xl_tiles[t][:], in_=xl_d[:, t])

            # Input-ring FIFO order tuned so the PE never starves at startup:
            # w0k0+x0 unlock MM #0 ~2us in, then each w0 chunk lands just
            # ahead of its consumer MM, so the PE ramps once and stays at
            # full clock. Bias rides last — it is only needed by the first
            # DVE add (~6us) and must not block the DMA device early.
            load_w(0, 0, wch=WCH0)
            load_x(0)
            for kc in range(WCH0, KO, WCH0):
                load_w(0, kc, wch=WCH0)
            load_x(1)
            # bias: one 4 KB row DMA + Pool-engine partition broadcast — keeps
            # the 512 KB replication entirely off the DMA device
            bias_row = singles.tile([1, OUT_F], f32)
            nc.scalar.dma_start(out=bias_row[:], in_=bias_d[:])
            nc.gpsimd.partition_broadcast(bias_sb[:], bias_row[:])
            for t in range(2, NT):
                load_x(t)
            for kc in range(0, KO, WCH):
                load_w(1, kc)
            if split:
                for kc in range(0, KO, WCH):
                    load_w(0, kc, lo=True)
                for kc in range(0, KO, WCH):
                    load_w(1, kc, lo=True)

            for ot in range(NO):
                for t in range(NT):
                    ps = psums.tile([P, OW], f32, name="ps", tag="ps")
                    groups = [(x_tiles[t], w_tiles[ot])]
                    if split:
                        groups += [(xl_tiles[t], w_tiles[ot]),
                                   (x_tiles[t], wl_tiles[ot])]
                    n_mm = len(groups) * KO
                    i = 0
                    for lhs_sb, rhs_sb in groups:
                        for ko in range(KO):
                            nc.tensor.matmul(
                                ps[:],
                                lhs_sb[:, ko],
                                rhs_sb[:, ko],
                                start=(i == 0),
                                stop=(i == n_mm - 1),
                            )
                            i += 1
                    if not skip_dve:
                        nc.vector.tensor_add(
                            o_tiles[t][ot][:],
                            ps[:],
                            bias_sb[:, ot * OW:(ot + 1) * OW],
                        )
                    if not (skip_out or skip_dve):
                        # per-half output DMA: first-half outs drain while the
                        # PE crunches the second half; tail is one 256 KB DMA
                        nc.scalar.dma_start(
                            out=out_d[t * P:(t + 1) * P, ot * OW:(ot + 1) * OW],
                            in_=o_tiles[t][ot][:],
                        )
    nc.compile()
    return nc


def _get_nc(mode):
    if mode not in _nc_cache:
        _nc_cache[mode] = _build(mode)
    return _nc_cache[mode]


def _pack(x, W, b, mode):
    """Shard + retile host-side. Returns in_maps for the 8 cores."""
    np_dt = np.float16 if mode in ("f16", "f16x3") else np.float32
    x = np.asarray(x, dtype=np.float32)
    W = np.asarray(W, dtype=np.float32)
    b = np.asarray(b, dtype=np.float32)

    # [c, t, bi, ko, ki] -> [c, ki, t, ko, bi]
    xs = x.reshape(N_CORES, NT, P, KO, P).transpose(0, 4, 1, 3, 2)
    # [ot, oi, ko, ki] -> [ki, ot, ko, oi]
    ws = W.reshape(NO, OW, KO, P).transpose(3, 0, 2, 1)
    bias = np.ascontiguousarray(b.reshape(1, OUT_F))

    xt = np.ascontiguousarray(xs).astype(np_dt)
    wt = np.ascontiguousarray(ws).astype(np_dt)
    maps = [{"xt": xt[c], "wt": wt, "bias": bias} for c in range(N_CORES)]
    if mode == "f16x3":
        xlo = (xs - xt.astype(np.float32)).astype(np_dt)
        wlo = (ws - wt.astype(np.float32)).astype(np_dt)
        for c in range(N_CORES):
            maps[c]["xl"] = np.ascontiguousarray(xlo[c])
            maps[c]["wl"] = wlo
    return maps


def _run(in_maps, mode, **kwargs):
    nc = _get_nc(mode)
    return run_bass_kernel_spmd(nc, in_maps, core_ids=list(range(N_CORES)), **kwargs)


def kernel(x, W, b):
    mode = MODE
    res = _run(_pack(x, W, b, mode), mode)
    out = np.concatenate([r["out"] for r in res.results], axis=0)
    return np.ascontiguousarray(out, dtype=np.float32)

